# revision 6
# baseline (speedup 1.0000x reference)
"""Single-head causal attention (B=8, T=2048, C=1024, H=64) on 8 TRN2 NeuronCores.

Strategy: pure data parallelism — batch element b runs on core b. Each core
computes, for its [T, C] slices q_b / k_b:

    Q = q_b @ Wq ; K = k_b @ Wk ; V = k_b @ Wv          (projections)
    S = Q @ K^T / sqrt(C), causal-masked ; P = exp(S)    (no max-subtract:
    out = (P @ V) / (P @ 1)                               S is well-scaled)

Device-side layout:
  * Host pre-transposes q/k to [C, T] and pre-blocks them [tb, p, c, t];
    contraction (C) lands on SBUF partitions, zero on-chip input transposes.
    k ships bf16 (feeds K and V; fp8 k fails the V-precision budget);
    q ships fp8-e4m3 and Wq ships fp8 pre-scaled by 64 (1/64 folded into
    the exp scale), so the Q projection runs in DoubleRow mode.
  * Warm-up matmuls run on a DVE-memset scratch tile (no identity DMA),
    so the PE HAM clock-gate ramp starts as soon as the preamble ends.
  * V is re-oriented [key, h] for the PV matmuls by an SBUF->SBUF xbar
    DMA transpose (one per 512-col block) — zero PE/DVE cost.  The ones
    column for softmax denominators sits at V1 col 64 (so the V cols stay
    32B-aligned for the xbar); PV output row 64 is the denominator.
  * Q projection uses a duplicated stationary [Wq | Wq], so Q^T comes out
    replicated on partition halves 0:64 / 64:128 in one pass.  K^T is
    likewise replicated to partitions 64:128 by a PE shift-matmul.
  * Scores run as ROW-TILED PAIRS: chunk for key-tile j uses PE rows 0:63,
    chunk j+1 rows 64:127; the two matmuls execute concurrently.  The B
    chunk lands at PSUM column wA (right after the A chunk), so one wide
    scalar-engine exp covers both with zero wasted columns.
  * The emission order keeps the scalar engine's exp rail dense: per block
    Q projects before KV, and DMA arrival order (cb, wq, q0, k0, q1, k1,
    q3, k2, q2, k3) is matched to the exp schedule (block-3 partial sums
    interleave after attn block 1).
  * Softmax denominators come free via the ones column in V1 (row 64 of
    the PV accumulator is P @ 1).  Outputs ship unnormalized [out^T ; l]
    fp32 via one DVE copy + DMA; the host divides.
"""

import numpy as np
import ml_dtypes

B, T, C, H = 8, 2048, 1024, 64
P = 128                  # SBUF partitions
CCH = C // P             # 8 contraction chunks
NJ = T // P              # 16 key tiles of 128
NB = T // 512            # 4 column blocks of 512
SCALE = float(C) ** -0.5
QS = 64.0                # fp8 Wq pre-scale (folded out via exp scale)
NWARM = 20               # HAM warm-up matmuls (scratch)

_cached = {}


def _build():
    import concourse.bass as bass
    import concourse.mybir as mybir
    import concourse.tile as tile
    from concourse import bacc


    dt = mybir.dt
    nc = bacc.Bacc("TRN2", target_bir_lowering=False, debug=False, num_devices=B)

    qT = nc.dram_tensor("qT", [NB, P, CCH, 512], dt.float8e4, kind="ExternalInput").ap()
    kT = nc.dram_tensor("kT", [NB, P, CCH, 512], dt.bfloat16, kind="ExternalInput").ap()
    wq = nc.dram_tensor("wq", [P, CCH, P], dt.float8e4, kind="ExternalInput").ap()
    # packed bf16 consts: [wkv (CCH chunks) | shift | causal mask] — one DMA
    cb = nc.dram_tensor("cb", [P, CCH + 2, P], dt.bfloat16, kind="ExternalInput").ap()
    # unnormalized [out^T ; l] per column block; host divides rows 0:64 by row 64
    out_t = nc.dram_tensor("out_t", [H + 1, T], dt.float32, kind="ExternalOutput").ap()
    # i-block 3 ships as 4 partial accumulations (host sums + normalizes)
    out3p = nc.dram_tensor("out3p", [4, H + 1, 512], dt.float32,
                           kind="ExternalOutput").ap()

    EXP = mybir.ActivationFunctionType.Exp
    DR = mybir.MatmulPerfMode.DoubleRow

    with tile.TileContext(nc) as tc:
        with (
            tc.tile_pool(name="consts", bufs=1) as consts,
            tc.tile_pool(name="inbuf", bufs=1) as inbuf,
            tc.tile_pool(name="proj", bufs=1) as proj,
            tc.tile_pool(name="projpsum", bufs=1, space="PSUM") as projpsum,
            tc.tile_pool(name="spsum", bufs=2, space="PSUM") as spsum,
            tc.tile_pool(name="opsum", bufs=2, space="PSUM") as opsum,
            tc.tile_pool(name="pbuf", bufs=3) as pbuf,
        ):
            # ---- constants / scratch ----------------------------------------
            scr_s = consts.tile([P, P], dt.bfloat16)     # warm-up matmul fodder
            cb_s = consts.tile([P, CCH + 2, P], dt.bfloat16)
            wq_s = consts.tile([P, CCH, P], dt.float8e4)
            wkv_s = cb_s[:, 0:CCH, :]
            nc.vector.memset(scr_s[:], 1.0)

            kT_s = inbuf.tile([P, NB, CCH, 512], dt.bfloat16)
            qT_s = inbuf.tile([P, NB, CCH, 512], dt.float8e4)
            KVT_s = proj.tile([P, T], dt.bfloat16)   # rows 0:64 K^T, 64:128 V^T
            QT_s = proj.tile([P, T], dt.bfloat16)    # Q^T duplicated both halves
            KTD_s = proj.tile([P, T], dt.bfloat16)   # rows 64:128 = K^T dup
            # V natural [key, h] at cols 0:64, ones col at 64, pad to 80
            V1_s = proj.tile([P, NJ, 80], dt.bfloat16)
            nc.vector.memset(V1_s[:, :, 64:65], 1.0)

            # ---- input DMAs: ordered to match the exp-rail schedule ---------
            def dma_k(tb):
                nc.sync.dma_start(out=kT_s[:, tb, 0:4], in_=kT[tb, :, 0:4])
                nc.sync.dma_start(out=kT_s[:, tb, 4:8], in_=kT[tb, :, 4:8])

            def dma_q(tb):
                nc.sync.dma_start(out=qT_s[:, tb], in_=qT[tb, :])

            nc.sync.dma_start(out=cb_s[:], in_=cb[:])
            nc.sync.dma_start(out=wq_s[:], in_=wq[:])
            dma_q(0)
            dma_k(0)
            dma_q(1)
            dma_k(1)
            dma_q(3)
            dma_k(2)
            dma_q(2)
            dma_k(3)

            # ---- HAM warm-up: scratch matmuls while the first DMAs stream ---
            for w in range(NWARM):
                wp = projpsum.tile([P, 512], dt.float32, tag="kv" if w % 2 else "q")
                nc.tensor.matmul(wp[:, 0:P], lhsT=scr_s[:], rhs=scr_s[:],
                                 start=True, stop=True)

            # ---- pipeline stages --------------------------------------------
            def proj_kv_mms(tb):
                """KV projection matmuls as 4 chunks of 2 (interleavable)."""
                KVp = projpsum.tile([P, 512], dt.float32, tag="kv")

                def chunk(c2):
                    def emit():
                        for c in (2 * c2, 2 * c2 + 1):
                            nc.tensor.matmul(KVp[:], lhsT=wkv_s[:, c, :],
                                             rhs=kT_s[:, tb, c, :],
                                             start=(c == 0), stop=(c == CCH - 1))
                    return emit
                return KVp, [chunk(c2) for c2 in range(CCH // 2)]

            def proj_kv(tb, KVp=None):
                """Finish one k-block projection: copy K^T/V^T, K^T dup shift,
                xbar-DMA V transpose.  KVp=None emits the KV matmuls here;
                otherwise they were interleaved earlier."""
                sl = slice(512 * tb, 512 * (tb + 1))
                if KVp is None:
                    KVp, chunks = proj_kv_mms(tb)
                    for ch in chunks:
                        ch()
                nc.vector.tensor_copy(out=KVT_s[:, sl], in_=KVp[:])
                # replicate K^T onto partitions 64:128 for row-tiled scores:
                # PE shift-matmul (out[64+i,:] = K^T[i,:]) + small DVE copy
                KDp = projpsum.tile([P, 512], dt.float32, tag="kv")
                nc.tensor.matmul(KDp[:], lhsT=cb_s[0:64, CCH, :],
                                 rhs=KVT_s[0:64, sl], start=True, stop=True)
                nc.vector.tensor_copy(out=KTD_s[64:128, sl], in_=KDp[64:128, :])
                # V -> [key, h] via SBUF->SBUF xbar transpose (off the PE).
                # Block 0 issues from the scalar queue (sync is mid input
                # stream and would deliver it too late); rest from sync.
                eng = nc.scalar if tb == 0 else nc.sync
                eng.dma_start_transpose(out=V1_s[:, 4 * tb:4 * tb + 4, 0:64],
                                        in_=KVT_s[64:128, sl])

            def proj_q(tb):
                """Project one 512-col block of q into Q^T (DoubleRow fp8)."""
                sl = slice(512 * tb, 512 * (tb + 1))
                Qp = projpsum.tile([P, 512], dt.float32, tag="q")
                for c2 in range(CCH // 2):
                    nc.tensor.matmul(Qp[:], lhsT=wq_s[:, 2 * c2:2 * c2 + 2, :],
                                     rhs=qT_s[:, tb, 2 * c2:2 * c2 + 2, :],
                                     perf_mode=DR,
                                     start=(c2 == 0), stop=(c2 == CCH // 2 - 1))
                nc.vector.tensor_copy(out=QT_s[:, sl], in_=Qp[:])

            def attn_pairs(ic, pairs, part_first, part_last, fillers=()):
                """Row-tiled score pairs + exp + PV accumulation for i-block ic.

                pairs: list of (jA, jB).  part_first/part_last bound the PSUM
                accumulation group for this call."""
                ilo = 512 * ic
                ihi = 512 * (ic + 1)
                OUTp = opsum.tile([H + 1, 512], dt.float32, tag="out")

                def emit_pv(pv, is_first, is_last):
                    jA, jB, wA, wB, loA, loB, ao, Pt = pv
                    nc.tensor.matmul(OUTp[:, loA - ilo:512],
                                     lhsT=V1_s[:, jA, 0:65],
                                     rhs=Pt[:, ao:512],
                                     start=is_first, stop=False)
                    nc.tensor.matmul(OUTp[:, loB - ilo:512],
                                     lhsT=V1_s[:, jB, 0:65],
                                     rhs=Pt[:, 512:512 + wB],
                                     start=False, stop=is_last)

                # scores run one pair AHEAD of PV so the in-order PE never
                # sits at a PV waiting for exp when the next S could run
                pend = None
                for pi, (jA, jB) in enumerate(pairs):
                    loA = max(P * jA, ilo)
                    loB = max(P * jB, ilo)
                    wA = ihi - loA
                    wB = ihi - loB
                    # A right-aligned against the bank-0 boundary, B at the
                    # start of bank 1: concurrent drains hit separate banks
                    # and the exp window [ao : 512+wB] is contiguous.
                    ao = 512 - wA
                    Sp = spsum.tile([P, 1024], dt.float32, tag="s")
                    nc.tensor.matmul(Sp[:, ao:512],
                                     lhsT=KVT_s[0:H, P * jA:P * (jA + 1)],
                                     rhs=QT_s[0:H, loA:loA + wA],
                                     start=True, stop=True)
                    nc.tensor.matmul(Sp[:, 512:512 + wB],
                                     lhsT=KTD_s[64:128, P * jB:P * (jB + 1)],
                                     rhs=QT_s[64:128, loB:loB + wB],
                                     start=True, stop=True)
                    Pt = pbuf.tile([P, 1024], dt.bfloat16, tag="p")
                    nc.scalar.activation(out=Pt[:, ao:512 + wB],
                                         in_=Sp[:, ao:512 + wB],
                                         func=EXP, scale=SCALE / QS)
                    if jA >= 4 * ic:
                        nc.vector.tensor_mul(Pt[:, ao:ao + P], Pt[:, ao:ao + P],
                                             cb_s[:, CCH + 1, :])
                    if jB >= 4 * ic:
                        nc.vector.tensor_mul(Pt[:, 512:512 + P],
                                             Pt[:, 512:512 + P],
                                             cb_s[:, CCH + 1, :])
                    if pi < len(fillers):
                        fillers[pi]()
                    if pend is not None:
                        emit_pv(pend, part_first and pi == 1, False)
                    pend = (jA, jB, wA, wB, loA, loB, ao, Pt)
                emit_pv(pend, part_first and len(pairs) == 1, part_last)
                return OUTp

            def attn_block(ic, fillers=()):
                """Full attention for i-block ic; unnormalized store via SBUF."""
                nj = 4 * ic + 4
                pairs = [(2 * p_, 2 * p_ + 1) for p_ in range(nj // 2)]
                OUTp = attn_pairs(ic, pairs, True, True, fillers)
                ob = pbuf.tile([H + 1, 512], dt.float32, tag="ob")
                nc.vector.tensor_copy(out=ob[:], in_=OUTp[:])
                nc.sync.dma_start(out=out_t[:, 512 * ic:512 * (ic + 1)],
                                  in_=ob[:])

            def attn3_part(pairs, pi, fillers=()):
                """Spread part of i-block 3: ship its partial sums; host adds."""
                OUTp = attn_pairs(3, pairs, True, True, fillers)
                ob = pbuf.tile([H + 1, 512], dt.float32, tag="ob")
                nc.vector.tensor_copy(out=ob[:], in_=OUTp[:])
                nc.sync.dma_start(out=out3p[pi], in_=ob[:])

            proj_q(0)
            proj_kv(0)
            attn_block(0)
            proj_q(1)
            proj_kv(1)
            attn_block(1)
            proj_q(3)
            attn3_part([(0, 1), (2, 3)], 0)
            attn3_part([(4, 5), (6, 7)], 1)
            proj_q(2)
            proj_kv(2)
            attn_block(2)
            attn3_part([(8, 9), (10, 11)], 2)
            proj_kv(3)
            attn3_part([(12, 13), (14, 15)], 3)

    nc.compile()
    return nc


def _get_nc():
    if "nc" not in _cached:
        _cached["nc"] = _build()
    return _cached["nc"]


def _block(xT, dtype):
    """[C, T] -> [NB, P, CCH, 512] so each 512-col block is contiguous."""
    return np.ascontiguousarray(
        xT.reshape(CCH, P, NB, 512).transpose(2, 1, 0, 3)).astype(dtype)


def _wblock(w, dtype):
    """[C, Hw] -> [P, CCH, Hw] contiguous (contraction chunks on partitions)."""
    return np.ascontiguousarray(
        w.reshape(CCH, P, w.shape[1]).transpose(1, 0, 2)).astype(dtype)


def _host_inputs(q, k, Wq, Wk, Wv):
    bf16 = ml_dtypes.bfloat16
    fp8 = ml_dtypes.float8_e4m3
    wq_h = _wblock(np.concatenate([Wq, Wq], axis=1) * QS, fp8)
    wkv_h = _wblock(np.concatenate([Wk, Wv], axis=1), bf16)
    dmask_h = np.triu(np.ones((P, P), dtype=np.float32)).astype(bf16)
    shf_h = np.zeros((P, P), dtype=np.float32)
    shf_h[np.arange(64), 64 + np.arange(64)] = 1.0   # out[64+i] = in[i]
    shf_h = shf_h.astype(bf16)
    cb_h = np.concatenate(
        [wkv_h, shf_h[:, None, :], dmask_h[:, None, :]], axis=1)
    in_maps = []
    for b in range(B):
        in_maps.append({
            "qT": _block(q[b].T, fp8),
            "kT": _block(k[b].T, bf16),
            "wq": wq_h,
            "cb": cb_h,
        })
    return in_maps


def _unshard(res_b):
    o = res_b["out_t"].copy()               # [H+1, T] f32: row H = l
    o[:, 1536:2048] = res_b["out3p"].sum(axis=0)
    return (o[0:H] / o[H:H + 1]).T          # [T, H]


def kernel(q, k, Wq, Wk, Wv):
    from concourse.bass_utils import run_bass_kernel_spmd

    nc = _get_nc()
    in_maps = _host_inputs(q, k, Wq, Wk, Wv)
    res = run_bass_kernel_spmd(nc, in_maps, list(range(B)))
    return np.stack([_unshard(res.results[b]) for b in range(B)]).astype(np.float32)


if __name__ == "__main__":
    rng = np.random.default_rng(0)
    q = rng.standard_normal((B, T, C)).astype(np.float32)
    k = rng.standard_normal((B, T, C)).astype(np.float32)
    Wq = (rng.standard_normal((C, H)) * 0.02).astype(np.float32)
    Wk = (rng.standard_normal((C, H)) * 0.02).astype(np.float32)
    Wv = (rng.standard_normal((C, H)) * 0.02).astype(np.float32)
    o = kernel(q, k, Wq, Wk, Wv)
    print("out", o.shape, o.dtype, float(np.abs(o).max()))


# revision 7
# speedup vs baseline: 1.0845x; 1.0845x over previous
"""Single-head causal attention (B=8, T=2048, C=1024, H=64) on 8 TRN2 NeuronCores.

Strategy: pure data parallelism — batch element b runs on core b. Each core
computes, for its [T, C] slices q_b / k_b:

    Q = q_b @ Wq ; K = k_b @ Wk ; V = k_b @ Wv          (projections)
    S = Q @ K^T / sqrt(C), causal-masked ; P = exp(S)    (no max-subtract:
    out = (P @ V) / (P @ 1)                               S is well-scaled)

Device-side layout:
  * Host pre-transposes q/k to [C, T] and pre-blocks them [tb, p, c, t];
    contraction (C) lands on SBUF partitions, zero on-chip input transposes.
    k ships bf16 (feeds K and V; fp8 k fails the V-precision budget);
    q ships fp8-e4m3 and Wq ships fp8 pre-scaled by 64 (1/64 folded into
    the exp scale), so the Q projection runs in DoubleRow mode.
  * Warm-up matmuls run on a DVE-memset scratch tile (no identity DMA),
    so the PE HAM clock-gate ramp starts as soon as the preamble ends.
  * V is re-oriented [key, h] for the PV matmuls by an SBUF->SBUF xbar
    DMA transpose (one per 512-col block) — zero PE/DVE cost.  The ones
    column for softmax denominators sits at V1 col 64 (so the V cols stay
    32B-aligned for the xbar); PV output row 64 is the denominator.
  * Q projection uses a duplicated stationary [Wq | Wq], so Q^T comes out
    replicated on partition halves 0:64 / 64:128 in one pass.  K^T is
    likewise replicated to partitions 64:128 by a PE shift-matmul.
  * Scores run as ROW-TILED PAIRS: chunk for key-tile j uses PE rows 0:63,
    chunk j+1 rows 64:127; the two matmuls execute concurrently.  The B
    chunk lands at PSUM column wA (right after the A chunk), so one wide
    scalar-engine exp covers both with zero wasted columns.
  * The emission order keeps the scalar engine's exp rail dense: per block
    Q projects before KV, and DMA arrival order (cb, wq, q0, k0, q1, k1,
    q3, k2, q2, k3) is matched to the exp schedule (block-3 partial sums
    interleave after attn block 1).
  * Softmax denominators come free via the ones column in V1 (row 64 of
    the PV accumulator is P @ 1).  Outputs ship unnormalized [out^T ; l]
    fp32 via one DVE copy + DMA; the host divides.
"""

import numpy as np
import ml_dtypes

B, T, C, H = 8, 2048, 1024, 64
P = 128                  # SBUF partitions
CCH = C // P             # 8 contraction chunks
NJ = T // P              # 16 key tiles of 128
NB = T // 512            # 4 column blocks of 512
SCALE = float(C) ** -0.5
QS = 64.0                # fp8 Wq pre-scale (folded out via exp scale)
NWARM = 12               # HAM warm-up matmuls (scratch)

_cached = {}


def _build():
    import concourse.bass as bass
    import concourse.mybir as mybir
    import concourse.tile as tile
    from concourse import bacc


    dt = mybir.dt
    nc = bacc.Bacc("TRN2", target_bir_lowering=False, debug=False, num_devices=B)

    qT = nc.dram_tensor("qT", [NB, P, CCH, 512], dt.float8e4, kind="ExternalInput").ap()
    kT = nc.dram_tensor("kT", [NB, P, CCH, 512], dt.bfloat16, kind="ExternalInput").ap()
    wq = nc.dram_tensor("wq", [P, CCH, P], dt.float8e4, kind="ExternalInput").ap()
    # packed bf16 consts: [wkv (CCH chunks) | shift | causal mask] — one DMA
    cb = nc.dram_tensor("cb", [P, CCH + 2, P], dt.bfloat16, kind="ExternalInput").ap()
    # unnormalized [out^T ; l] per column block; host divides rows 0:64 by row 64
    out_t = nc.dram_tensor("out_t", [H + 1, T], dt.float32, kind="ExternalOutput").ap()
    # i-block 3 ships as 4 partial accumulations (host sums + normalizes)
    out3p = nc.dram_tensor("out3p", [4, H + 1, 512], dt.float32,
                           kind="ExternalOutput").ap()

    EXP = mybir.ActivationFunctionType.Exp
    DR = mybir.MatmulPerfMode.DoubleRow

    with tile.TileContext(nc) as tc:
        with (
            tc.tile_pool(name="consts", bufs=1) as consts,
            tc.tile_pool(name="inbuf", bufs=1) as inbuf,
            tc.tile_pool(name="proj", bufs=1) as proj,
            tc.tile_pool(name="projpsum", bufs=1, space="PSUM") as projpsum,
            tc.tile_pool(name="spsum", bufs=2, space="PSUM") as spsum,
            tc.tile_pool(name="opsum", bufs=2, space="PSUM") as opsum,
            tc.tile_pool(name="pbuf", bufs=3) as pbuf,
        ):
            # ---- constants / scratch ----------------------------------------
            scr_s = consts.tile([P, 512], dt.bfloat16)   # warm-up matmul fodder
            cb_s = consts.tile([P, CCH + 2, P], dt.bfloat16)
            wq_s = consts.tile([P, CCH, P], dt.float8e4)
            wkv_s = cb_s[:, 0:CCH, :]
            nc.vector.memset(scr_s[:], 1.0)

            kT_s = inbuf.tile([P, NB, CCH, 512], dt.bfloat16)
            qT_s = inbuf.tile([P, NB, CCH, 512], dt.float8e4)
            KVT_s = proj.tile([P, T], dt.bfloat16)   # rows 0:64 K^T, 64:128 V^T
            QT_s = proj.tile([P, T], dt.bfloat16)    # Q^T duplicated both halves
            KTD_s = proj.tile([P, T], dt.bfloat16)   # rows 64:128 = K^T dup
            # V natural [key, h] at cols 0:64, ones col at 64, pad to 80
            V1_s = proj.tile([P, NJ, 80], dt.bfloat16)
            nc.vector.memset(V1_s[:, :, 64:65], 1.0)

            # ---- input DMAs: ordered to match the exp-rail schedule ---------
            def dma_k(tb):
                nc.sync.dma_start(out=kT_s[:, tb, 0:4], in_=kT[tb, :, 0:4])
                nc.sync.dma_start(out=kT_s[:, tb, 4:8], in_=kT[tb, :, 4:8])

            def dma_q(tb):
                nc.sync.dma_start(out=qT_s[:, tb], in_=qT[tb, :])

            nc.sync.dma_start(out=cb_s[:], in_=cb[:])
            nc.sync.dma_start(out=wq_s[:], in_=wq[:])
            dma_q(0)
            dma_k(0)
            dma_q(1)
            dma_k(1)
            dma_q(3)
            dma_k(2)
            dma_q(2)
            dma_k(3)

            # ---- HAM warm-up: scratch matmuls while the first DMAs stream ---
            for w in range(NWARM):
                wp = projpsum.tile([P, 512], dt.float32, tag="kv" if w % 2 else "q")
                nc.tensor.matmul(wp[:], lhsT=scr_s[:, 0:P], rhs=scr_s[:],
                                 start=True, stop=True)

            # ---- pipeline stages --------------------------------------------
            def proj_kv_mms(tb):
                """KV projection matmuls as 4 chunks of 2 (interleavable)."""
                KVp = projpsum.tile([P, 512], dt.float32, tag="kv")

                def chunk(c2):
                    def emit():
                        for c in (2 * c2, 2 * c2 + 1):
                            nc.tensor.matmul(KVp[:], lhsT=wkv_s[:, c, :],
                                             rhs=kT_s[:, tb, c, :],
                                             start=(c == 0), stop=(c == CCH - 1))
                    return emit
                return KVp, [chunk(c2) for c2 in range(CCH // 2)]

            def proj_kv(tb, KVp=None):
                """Finish one k-block projection: copy K^T/V^T, K^T dup shift,
                xbar-DMA V transpose.  KVp=None emits the KV matmuls here;
                otherwise they were interleaved earlier."""
                sl = slice(512 * tb, 512 * (tb + 1))
                if KVp is None:
                    KVp, chunks = proj_kv_mms(tb)
                    for ch in chunks:
                        ch()
                nc.vector.tensor_copy(out=KVT_s[0:64, sl], in_=KVp[0:64, :])
                # replicate K^T onto partitions 64:128 for row-tiled scores:
                # PE shift-matmul (out[64+i,:] = K^T[i,:]) + small DVE copy
                KDp = projpsum.tile([P, 512], dt.float32, tag="kv")
                nc.tensor.matmul(KDp[:], lhsT=cb_s[0:64, CCH, :],
                                 rhs=KVT_s[0:64, sl], start=True, stop=True)
                nc.vector.tensor_copy(out=KTD_s[64:128, sl], in_=KDp[64:128, :])
                nc.vector.tensor_copy(out=KVT_s[64:128, sl], in_=KVp[64:128, :])
                # V -> [key, h] via SBUF->SBUF xbar transpose (off the PE).
                # Block 0 issues from the scalar queue (sync is mid input
                # stream and would deliver it too late); rest from sync.
                eng = nc.scalar if tb == 0 else nc.sync
                eng.dma_start_transpose(out=V1_s[:, 4 * tb:4 * tb + 4, 0:64],
                                        in_=KVT_s[64:128, sl])

            def proj_q(tb):
                """Project one 512-col block of q into Q^T (DoubleRow fp8)."""
                sl = slice(512 * tb, 512 * (tb + 1))
                Qp = projpsum.tile([P, 512], dt.float32, tag="q")
                for c2 in range(CCH // 2):
                    nc.tensor.matmul(Qp[:], lhsT=wq_s[:, 2 * c2:2 * c2 + 2, :],
                                     rhs=qT_s[:, tb, 2 * c2:2 * c2 + 2, :],
                                     perf_mode=DR,
                                     start=(c2 == 0), stop=(c2 == CCH // 2 - 1))
                nc.vector.tensor_copy(out=QT_s[:, sl], in_=Qp[:])

            def attn_pairs(ic, pairs, part_first, part_last, fillers=()):
                """Row-tiled score pairs + exp + PV accumulation for i-block ic.

                pairs: list of (jA, jB).  part_first/part_last bound the PSUM
                accumulation group for this call."""
                ilo = 512 * ic
                ihi = 512 * (ic + 1)
                OUTp = opsum.tile([H + 1, 512], dt.float32, tag="out")

                def emit_pv(pv, is_first, is_last):
                    jA, jB, wA, wB, loA, loB, ao, Pt = pv
                    nc.tensor.matmul(OUTp[:, loA - ilo:512],
                                     lhsT=V1_s[:, jA, 0:65],
                                     rhs=Pt[:, ao:512],
                                     start=is_first, stop=False)
                    nc.tensor.matmul(OUTp[:, loB - ilo:512],
                                     lhsT=V1_s[:, jB, 0:65],
                                     rhs=Pt[:, 512:512 + wB],
                                     start=False, stop=is_last)

                # scores run one pair AHEAD of PV so the in-order PE never
                # sits at a PV waiting for exp when the next S could run
                pend = None
                for pi, (jA, jB) in enumerate(pairs):
                    loA = max(P * jA, ilo)
                    loB = max(P * jB, ilo)
                    wA = ihi - loA
                    wB = ihi - loB
                    # A right-aligned against the bank-0 boundary, B at the
                    # start of bank 1: concurrent drains hit separate banks
                    # and the exp window [ao : 512+wB] is contiguous.
                    ao = 512 - wA
                    Sp = spsum.tile([P, 1024], dt.float32, tag="s")
                    nc.tensor.matmul(Sp[:, ao:512],
                                     lhsT=KVT_s[0:H, P * jA:P * (jA + 1)],
                                     rhs=QT_s[0:H, loA:loA + wA],
                                     start=True, stop=True)
                    nc.tensor.matmul(Sp[:, 512:512 + wB],
                                     lhsT=KTD_s[64:128, P * jB:P * (jB + 1)],
                                     rhs=QT_s[64:128, loB:loB + wB],
                                     start=True, stop=True)
                    Pt = pbuf.tile([P, 1024], dt.bfloat16, tag="p")
                    nc.scalar.activation(out=Pt[:, ao:512 + wB],
                                         in_=Sp[:, ao:512 + wB],
                                         func=EXP, scale=SCALE / QS)
                    if jA >= 4 * ic:
                        nc.vector.tensor_mul(Pt[:, ao:ao + P], Pt[:, ao:ao + P],
                                             cb_s[:, CCH + 1, :])
                    if jB >= 4 * ic:
                        nc.vector.tensor_mul(Pt[:, 512:512 + P],
                                             Pt[:, 512:512 + P],
                                             cb_s[:, CCH + 1, :])
                    if pi < len(fillers):
                        fillers[pi]()
                    if pend is not None:
                        emit_pv(pend, part_first and pi == 1, False)
                    pend = (jA, jB, wA, wB, loA, loB, ao, Pt)
                emit_pv(pend, part_first and len(pairs) == 1, part_last)
                return OUTp

            def attn_block(ic, fillers=()):
                """Full attention for i-block ic; unnormalized store via SBUF."""
                nj = 4 * ic + 4
                pairs = [(2 * p_, 2 * p_ + 1) for p_ in range(nj // 2)]
                OUTp = attn_pairs(ic, pairs, True, True, fillers)
                ob = pbuf.tile([H + 1, 512], dt.float32, tag="ob")
                nc.vector.tensor_copy(out=ob[:], in_=OUTp[:])
                nc.sync.dma_start(out=out_t[:, 512 * ic:512 * (ic + 1)],
                                  in_=ob[:])

            def attn3_part(pairs, pi, fillers=()):
                """Spread part of i-block 3: ship its partial sums; host adds."""
                OUTp = attn_pairs(3, pairs, True, True, fillers)
                ob = pbuf.tile([H + 1, 512], dt.float32, tag="ob")
                nc.vector.tensor_copy(out=ob[:], in_=OUTp[:])
                nc.sync.dma_start(out=out3p[pi], in_=ob[:])

            proj_q(0)
            proj_kv(0)
            attn_block(0)
            proj_q(1)
            proj_kv(1)
            attn_block(1)
            proj_q(3)
            attn3_part([(0, 1), (2, 3)], 0)
            attn3_part([(4, 5), (6, 7)], 1)
            proj_q(2)
            proj_kv(2)
            attn_block(2)
            attn3_part([(8, 9), (10, 11)], 2)
            proj_kv(3)
            attn3_part([(12, 13), (14, 15)], 3)

    nc.compile()
    return nc


def _get_nc():
    if "nc" not in _cached:
        _cached["nc"] = _build()
    return _cached["nc"]


def _block(xT, dtype):
    """[C, T] -> [NB, P, CCH, 512] so each 512-col block is contiguous."""
    return np.ascontiguousarray(
        xT.reshape(CCH, P, NB, 512).transpose(2, 1, 0, 3)).astype(dtype)


def _wblock(w, dtype):
    """[C, Hw] -> [P, CCH, Hw] contiguous (contraction chunks on partitions)."""
    return np.ascontiguousarray(
        w.reshape(CCH, P, w.shape[1]).transpose(1, 0, 2)).astype(dtype)


def _host_inputs(q, k, Wq, Wk, Wv):
    bf16 = ml_dtypes.bfloat16
    fp8 = ml_dtypes.float8_e4m3
    wq_h = _wblock(np.concatenate([Wq, Wq], axis=1) * QS, fp8)
    wkv_h = _wblock(np.concatenate([Wk, Wv], axis=1), bf16)
    dmask_h = np.triu(np.ones((P, P), dtype=np.float32)).astype(bf16)
    shf_h = np.zeros((P, P), dtype=np.float32)
    shf_h[np.arange(64), 64 + np.arange(64)] = 1.0   # out[64+i] = in[i]
    shf_h = shf_h.astype(bf16)
    cb_h = np.concatenate(
        [wkv_h, shf_h[:, None, :], dmask_h[:, None, :]], axis=1)
    in_maps = []
    for b in range(B):
        in_maps.append({
            "qT": _block(q[b].T, fp8),
            "kT": _block(k[b].T, bf16),
            "wq": wq_h,
            "cb": cb_h,
        })
    return in_maps


def _unshard(res_b):
    o = res_b["out_t"].copy()               # [H+1, T] f32: row H = l
    o[:, 1536:2048] = res_b["out3p"].sum(axis=0)
    return (o[0:H] / o[H:H + 1]).T          # [T, H]


def kernel(q, k, Wq, Wk, Wv):
    from concourse.bass_utils import run_bass_kernel_spmd

    nc = _get_nc()
    in_maps = _host_inputs(q, k, Wq, Wk, Wv)
    res = run_bass_kernel_spmd(nc, in_maps, list(range(B)))
    return np.stack([_unshard(res.results[b]) for b in range(B)]).astype(np.float32)


if __name__ == "__main__":
    rng = np.random.default_rng(0)
    q = rng.standard_normal((B, T, C)).astype(np.float32)
    k = rng.standard_normal((B, T, C)).astype(np.float32)
    Wq = (rng.standard_normal((C, H)) * 0.02).astype(np.float32)
    Wk = (rng.standard_normal((C, H)) * 0.02).astype(np.float32)
    Wv = (rng.standard_normal((C, H)) * 0.02).astype(np.float32)
    o = kernel(q, k, Wq, Wk, Wv)
    print("out", o.shape, o.dtype, float(np.abs(o).max()))
